# revision 1
# baseline (speedup 1.0000x reference)
"""Bass/Tile TRN2 kernel for nn_Attn (general-score attention over encoder outputs).

Math: for each batch sample b
    energies[s] = h[b] . (W @ enc[b,s] + bias)          # reference form
               = enc[b,s] . (h[b] @ W) + h[b].bias      # associativity
Softmax is shift-invariant, so the constant h[b].bias drops out entirely
(masked positions are forced to -1e10 in both forms).  This turns the
[B,S,D]x[D,D] matmul into a [B,D]x[D,D] matmul plus a per-row dot product,
making the kernel memory-bound on streaming encoder_outputs once.

Sharding: data-parallel over batch, 8 samples per core on 8 cores.
"""

import os
import sys

if "/opt/trn_rl_repo" not in sys.path:
    sys.path.insert(0, "/opt/trn_rl_repo")

STAGE = os.environ.get("K_STAGE", "full")

from contextlib import ExitStack

import numpy as np

import concourse.bass as bass
import concourse.masks as masks
import concourse.mybir as mybir
import concourse.tile as tile
from concourse import bacc, bass_utils

B, S, D = 64, 2048, 1024
NCORES = 8
BL = B // NCORES          # samples per core
P = 128                   # partitions
NT = S // P               # 16 s-chunks per sample
KC = D // P               # 8 contraction chunks of 128
F32 = mybir.dt.float32
BF16 = mybir.dt.bfloat16
AX = mybir.AxisListType
OP = mybir.AluOpType
ACTF = mybir.ActivationFunctionType
NEG_BIG = -1.0e10


def _emit(tc, ctx, hid, enc, msk, w, out):
    nc = tc.nc

    consts = ctx.enter_context(tc.tile_pool(name="consts", bufs=1))
    encp = ctx.enter_context(tc.tile_pool(name="encp", bufs=12))
    ebp = ctx.enter_context(tc.tile_pool(name="ebp", bufs=2 * NT))
    small = ctx.enter_context(tc.tile_pool(name="small", bufs=2))
    junkp = ctx.enter_context(tc.tile_pool(name="junkp", bufs=2))
    ubp = ctx.enter_context(tc.tile_pool(name="ubp", bufs=2))
    csbp = ctx.enter_context(tc.tile_pool(name="csbp", bufs=1))
    psA = ctx.enter_context(tc.tile_pool(name="psA", bufs=2, space="PSUM"))
    psC = ctx.enter_context(tc.tile_pool(name="psC", bufs=1, space="PSUM"))
    psS = ctx.enter_context(tc.tile_pool(name="psS", bufs=2, space="PSUM"))

    # --- constants ---
    identity = consts.tile([P, P], F32, tag="identity")
    masks.make_identity(nc, identity[:])
    ones_row = consts.tile([1, P], F32, tag="ones_row")
    nc.vector.memset(ones_row[:], 1.0)
    ones_sq = consts.tile([P, P], F32, tag="ones_sq")
    nc.vector.memset(ones_sq[:], 1.0)

    # --- setup: u = h @ W  (u[b,d] = sum_e h[b,e] W[e,d]) ---
    w_tiles = []
    for k in range(KC):
        wt = consts.tile([P, D], F32, tag=f"w{k}")
        nc.sync.dma_start(wt[:], w[k * P:(k + 1) * P, :])
        w_tiles.append(wt)
    # hT chunks: [128e, BL] slices of hidden transposed
    hid_r = hid.rearrange("b (k p) -> k p b", p=P)
    ht_tiles = []
    for k in range(KC):
        ht = consts.tile([P, BL], F32, tag=f"ht{k}")
        nc.sync.dma_start(ht[:], hid_r[k])
        ht_tiles.append(ht)

    u_ps = psA.tile([BL, D], F32, tag="ub")
    for h in range(2):
        for k in range(KC):
            nc.tensor.matmul(
                u_ps[:, h * 512:(h + 1) * 512],
                lhsT=ht_tiles[k][:],
                rhs=w_tiles[k][:, h * 512:(h + 1) * 512],
                start=(k == 0),
                stop=(k == KC - 1),
            )
    u_sb = consts.tile([BL, D], F32, tag="u_sb")
    nc.vector.tensor_copy(u_sb[:, 0:512], u_ps[:, 0:512])
    nc.vector.tensor_copy(u_sb[:, 512:1024], u_ps[:, 512:1024])
    # row-selector matrix: sel[:, j*128:(j+1)*128] is [BL, 128] with row j all
    # ones -> matmul sel_j.T @ u_sb broadcasts u row j to all 128 partitions
    sel = consts.tile([BL, BL * P], F32, tag="sel")
    nc.gpsimd.memset(sel[:], 1.0)
    # keep 1.0 only where j*128 <= y < (j+1)*128 on partition j
    nc.gpsimd.affine_select(
        out=sel[:], in_=sel[:], pattern=[[1, BL * P]], channel_multiplier=-P,
        base=0, compare_op=OP.is_ge, fill=0.0,
    )
    nc.gpsimd.affine_select(
        out=sel[:], in_=sel[:], pattern=[[-1, BL * P]], channel_multiplier=P,
        base=P - 1, compare_op=OP.is_ge, fill=0.0,
    )

    enc_r = enc.rearrange("b (t p) d -> b t p d", p=P)
    msk_r = msk.rearrange("b (t p) -> b p t", p=P)

    # initial loads for sample 0
    et_tiles = [None] * BL
    mt_tiles = [None] * BL
    ub_tiles = [None] * BL

    def load_sample(j):
        et = []
        for t in range(NT):
            e1 = encp.tile([P, D], F32, tag="enc")
            nc.sync.dma_start(e1[:], enc_r[j, t])
            et.append(e1)
        mt = small.tile([P, NT], F32, tag="mask")
        nc.sync.dma_start(mt[:], msk_r[j])
        et_tiles[j] = et
        mt_tiles[j] = mt

    def make_ub(j):
        # broadcast u[j,:] to all 128 partitions via selector matmul, then
        # evict to SBUF (per-bank copies on the otherwise-idle ACT engine)
        ub = psA.tile([P, D], F32, tag="ub")
        for h in range(2):
            nc.tensor.matmul(
                ub[:, h * 512:(h + 1) * 512],
                lhsT=sel[:, j * P:(j + 1) * P],
                rhs=u_sb[:, h * 512:(h + 1) * 512],
                start=True,
                stop=True,
            )
        ub_sb = ubp.tile([P, D], F32, tag="ub_sb")
        for h in range(2):
            nc.scalar.copy(ub_sb[:, h * 512:(h + 1) * 512],
                           ub[:, h * 512:(h + 1) * 512])
        ub_tiles[j] = ub_sb

    load_sample(0)
    make_ub(0)

    pending = None  # deferred (cps, j) eviction from previous sample

    def evict(pend):
        cps, jj = pend
        csb = csbp.tile([1, D], F32, tag="csb")
        for h in range(2):
            nc.scalar.copy(csb[:, h * 512:(h + 1) * 512],
                           cps[:, h * 512:(h + 1) * 512])
        nc.sync.dma_start(out[jj:jj + 1, :], csb[:])

    for j in range(BL):
        et, mt, ub = et_tiles[j], mt_tiles[j], ub_tiles[j]

        # prefetch next sample's tiles
        if j + 1 < BL:
            load_sample(j + 1)

        # --- phase 1: energies[s] = enc[s,:] . u ---
        # DVE elementwise product, free-dim sum via ACT copy-with-accumulate.
        # Also cast each f32 chunk to bf16 (for the 1-cycle/row context
        # matmul), split between DVE and ACT to balance engine load; the f32
        # tile is released right after -> small encp pool suffices.
        e_col = small.tile([P, NT], F32, tag="e_col")
        eb = []
        for t in range(NT):
            junk = junkp.tile([P, D], F32, tag="junk")
            nc.vector.tensor_mul(junk[:], et[t][:], ub[:])
            nc.scalar.activation(
                junk[:], junk[:], ACTF.Copy,
                accum_out=e_col[:, t:t + 1],
            )
            eb_t = ebp.tile([P, D], BF16, tag="eb")
            if t % 2 == 0:
                nc.vector.tensor_copy(eb_t[:], et[t][:])
            else:
                nc.scalar.copy(eb_t[:], et[t][:])
            eb.append(eb_t)

        # previous sample's context eviction (lands after this sample's
        # ACT accumulate block -> never stalls the ACT stream on PE ctx)
        if pending is not None:
            evict(pending)
            pending = None

        # next sample's u broadcast (PE, before this sample's context matmuls)
        if j + 1 < BL:
            make_ub(j + 1)

        # --- phase 2: mask + softmax over all 2048 positions ---
        e2 = small.tile([P, NT], F32, tag="e2")
        nc.vector.tensor_mul(e2[:], e_col[:], mt[:])
        zm = small.tile([P, NT], F32, tag="zm")
        nc.vector.tensor_scalar(
            out=zm[:], in0=e2[:], scalar1=0.0, scalar2=NEG_BIG,
            op0=OP.is_equal, op1=OP.mult,
        )
        e3 = small.tile([P, NT], F32, tag="e3")
        nc.vector.tensor_add(e3[:], e2[:], zm[:])

        # global max via two PE transposes + free-dim reductions
        e3t = psS.tile([NT, P], F32, tag="psm")
        nc.tensor.transpose(e3t[:], e3[:], identity[:])
        m16 = small.tile([NT, 1], F32, tag="m16")
        nc.vector.tensor_reduce(m16[:], e3t[:], axis=AX.X, op=OP.max)
        m16t = psS.tile([1, NT], F32, tag="psm")
        nc.tensor.transpose(m16t[:], m16[:], identity[0:NT, 0:NT])
        mneg = small.tile([1, 1], F32, tag="mneg")
        nc.vector.tensor_reduce(mneg[:], m16t[:], axis=AX.X, op=OP.max, negate=True)
        mb_ps = psS.tile([P, 1], F32, tag="psm")
        nc.tensor.matmul(mb_ps[:], lhsT=ones_row[:], rhs=mneg[:], start=True, stop=True)
        mb_sb = small.tile([P, 1], F32, tag="mb_sb")
        nc.vector.tensor_copy(mb_sb[:], mb_ps[:])

        # exp(e3 - max) and per-partition row sums, fused on ACT
        attn = small.tile([P, NT], F32, tag="attn")
        s128 = small.tile([P, 1], F32, tag="s128")
        nc.scalar.activation(
            attn[:], e3[:], ACTF.Exp, bias=mb_sb[:], scale=1.0, accum_out=s128[:],
        )
        # total = sum over partitions, broadcast to all 128 (all-ones lhsT),
        # then per-partition reciprocal and normalize attn weights
        ssum = psS.tile([P, 1], F32, tag="psm")
        nc.tensor.matmul(ssum[:], lhsT=ones_sq[:], rhs=s128[:], start=True, stop=True)
        rinv = small.tile([P, 1], F32, tag="rinv")
        nc.vector.reciprocal(rinv[:], ssum[:])
        nc.vector.tensor_scalar_mul(attn[:], attn[:], rinv[:])

        if STAGE == "nocontext":
            out_r = out.rearrange("b (x p) -> b p x", p=P)
            nc.sync.dma_start(out_r[j, :, 0:8], attn[:, 0:8])
            continue

        # --- phase 3: context = (attn_unnorm @ enc) * rinv ---
        # bf16 single-pass matmuls (4x faster than fp32 on the PE); energies
        # and softmax stay fp32, only the weighted average runs bf16 (~5e-3)
        attn_bf = small.tile([P, NT], BF16, tag="attn_bf")
        nc.vector.tensor_copy(attn_bf[:], attn[:])
        cps = psC.tile([1, D], F32, tag="ctx")
        for h in range(2):
            for t in range(NT):
                nc.tensor.matmul(
                    cps[:, h * 512:(h + 1) * 512],
                    lhsT=attn_bf[:, t:t + 1],
                    rhs=eb[t][:, h * 512:(h + 1) * 512],
                    start=(t == 0),
                    stop=(t == NT - 1),
                )
        pending = (cps, j)

    if STAGE != "nocontext" and pending is not None:
        evict(pending)


def build_module():
    nc = bacc.Bacc("TRN2", target_bir_lowering=False, debug=False)
    hid = nc.dram_tensor("hid", [BL, D], F32, kind="ExternalInput").ap()
    enc = nc.dram_tensor("enc", [BL, S, D], F32, kind="ExternalInput").ap()
    msk = nc.dram_tensor("msk", [BL, S], F32, kind="ExternalInput").ap()
    w = nc.dram_tensor("w", [D, D], F32, kind="ExternalInput").ap()
    out = nc.dram_tensor("out", [BL, D], F32, kind="ExternalOutput").ap()
    with tile.TileContext(nc) as tc:
        with ExitStack() as ctx:
            _emit(tc, ctx, hid, enc, msk, w, out)
    nc.compile()
    return nc


_nc_cache = None


def kernel_with_results(hidden, encoder_outputs, attn_mask, W, b, **run_kwargs):
    global _nc_cache
    if _nc_cache is None:
        _nc_cache = build_module()
    nc = _nc_cache
    hidden = np.ascontiguousarray(np.asarray(hidden, dtype=np.float32))
    encoder_outputs = np.ascontiguousarray(np.asarray(encoder_outputs, dtype=np.float32))
    attn_mask = np.ascontiguousarray(np.asarray(attn_mask, dtype=np.float32))
    W = np.ascontiguousarray(np.asarray(W, dtype=np.float32))
    in_maps = []
    for c in range(NCORES):
        sl = slice(c * BL, (c + 1) * BL)
        in_maps.append({
            "hid": np.ascontiguousarray(hidden[0, sl]),
            "enc": np.ascontiguousarray(encoder_outputs[sl]),
            "msk": np.ascontiguousarray(attn_mask[sl]),
            "w": W,
        })
    res = bass_utils.run_bass_kernel_spmd(
        nc, in_maps, core_ids=list(range(NCORES)), **run_kwargs
    )
    out = np.concatenate([r["out"] for r in res.results], axis=0)
    return out, res


def kernel(**inputs):
    out, _ = kernel_with_results(**inputs)
    return out

